# revision 22
# baseline (speedup 1.0000x reference)
"""Causal multi-head attention with RoPE on 8 Trainium2 NeuronCores.

Problem: B=2, L=2048, D_MODEL=1024, N_HEADS=16, D_K=64, theta=10000.
Sharding: data parallel on batch (2) x tensor parallel on heads (4 groups of
4 heads) = 8 cores. Each core computes its 4 heads' attention plus a partial
output projection; partials are summed on the host (Megatron row-parallel).

v3 design (vs v2 baseline at 218us cost-model):
- All activations/weights in bf16 (DMA halved); Q/K additionally stored as
  fp8e4 in [128, 2(T/B plane), L] layout so each scores matmul is a single
  DoubleRow instruction per head per kv-tile (4x fewer PE cycles than the
  fp32r K=32 pairs of v2).
- Chunks of CH=256 q columns; scoresT strips [kv128, 4h*256] ping-pong in
  PSUM so the PE never waits on the exp of the previous tile.
- One exp ACT instruction per kv-tile over the whole strip (exact causal
  column slicing; bf16 out); DVE multiplies the 128-wide diagonal block by a
  lower-tri mask.
- AV is "swapped": out[q128, 65] per (head, qtile) with exp'd scores as the
  stationary and V (with an appended ones column = softmax denominator) as
  the 65-wide moving operand; accumulated over kv tiles qt-major so only the
  tiles below the diagonal are touched.
- Normalization is a DVE per-partition tensor_scalar multiply by 1/denom;
  the normalized [q,128] head-pair block is transposed to ho[d,q] layout by
  the DMA engine's hardware transpose (no PE/DVE cost).
- Output projection per q-tile follows immediately; PSUM->SBUF copies run on
  the Pool engine; DRAM output is bf16, host sums the 4 head-group partials
  in f32.
"""
import numpy as np
from contextlib import ExitStack

import concourse.bacc as bacc
import concourse.bass as bass
import concourse.mybir as mybir
import concourse.tile as tile
from concourse._compat import with_exitstack
from concourse.bass_utils import run_bass_kernel_spmd

F32 = mybir.dt.float32
BF16 = mybir.dt.bfloat16
FP8 = mybir.dt.float8e4

B, L, DM, NH, DK = 2, 2048, 1024, 16, 64
HPC = 4              # heads per core
THETA = 10000.0
CH = 256             # q chunk width
NCH = L // CH        # 8 chunks
NT = L // 128        # 16 kv tiles

_cache = {}
PHASE_RANGES = []   # (start_id, end_id, label) for trace attribution


def _track(nc, label, fn):
    def wrapped(*a, **k):
        s = nc.next_id()
        r = fn(*a, **k)
        PHASE_RANGES.append((s, nc.next_id(), f"{label}{a[:2]}"))
        return r
    return wrapped


@with_exitstack
def _attn_kernel(ctx: ExitStack, tc: tile.TileContext, outs, ins):
    nc = tc.nc
    xt, wq, wv, wo = ins["xt"], ins["wq"], ins["wv"], ins["wo"]
    cs, sn = ins["cs"], ins["sn"]
    out = outs["out"]
    AF = mybir.ActivationFunctionType
    DR = mybir.MatmulPerfMode.DoubleRow

    consts = ctx.enter_context(tc.tile_pool(name="consts", bufs=1))
    persist = ctx.enter_context(tc.tile_pool(name="persist", bufs=1))
    ps = ctx.enter_context(tc.tile_pool(name="ps", bufs=1, space="PSUM"))
    epool = ctx.enter_context(tc.tile_pool(name="epool", bufs=1))
    ropet = ctx.enter_context(tc.tile_pool(name="ropet", bufs=2))
    npool = ctx.enter_context(tc.tile_pool(name="npool", bufs=4))
    rpool = ctx.enter_context(tc.tile_pool(name="rpool", bufs=2))
    opool = ctx.enter_context(tc.tile_pool(name="opool", bufs=4))

    # ---- weights (bf16) + first xt stripes; d=0 pair first so the proj
    # accumulation chain starts as soon as each pair lands ----
    wq_sb, xt_sb = [], []
    for d in range(8):
        t_wq = consts.tile([128, 512], BF16, tag=f"wq{d}")
        nc.sync.dma_start(t_wq, wq[128 * d:128 * d + 128, :])
        wq_sb.append(t_wq)
        t_x = persist.tile([128, L], BF16, tag=f"xt{d}")
        nc.sync.dma_start(t_x[:, 0:2 * CH], xt[128 * d:128 * d + 128, 0:2 * CH])
        xt_sb.append(t_x)
    cs_sb = persist.tile([128, L], BF16)
    sn_sb = persist.tile([128, L], BF16)
    nc.sync.dma_start(cs_sb[:, 0:2 * CH], cs[:, 0:2 * CH])
    nc.sync.dma_start(sn_sb[:, 0:2 * CH], sn[:, 0:2 * CH])
    wv_sb, wo_sb = [], []
    for d in range(8):
        t_wv = consts.tile([128, 256], BF16, tag=f"wv{d}")
        nc.sync.dma_start(t_wv, wv[128 * d:128 * d + 128, :])
        wv_sb.append(t_wv)
    for j in range(2):
        t_wo = consts.tile([128, DM], BF16, tag=f"wo{j}")
        nc.sync.dma_start(t_wo, wo[128 * j:128 * j + 128, :])
        wo_sb.append(t_wo)

    # lower-triangular keep mask (keep iff q_local >= kv_local)
    tri = consts.tile([128, 128], BF16)
    nc.vector.memset(tri, 1.0)
    nc.gpsimd.affine_select(tri, tri, pattern=[[1, 128]],
                            compare_op=mybir.AluOpType.is_ge, fill=0.0,
                            base=0, channel_multiplier=-1)

    # persistent activations
    q8 = persist.tile([128, 2 * L], FP8, tag="q8")   # [4h*32, (T|B), L]
    k8 = persist.tile([128, 2 * L], FP8, tag="k8")
    q8p = q8[:].rearrange("p (two l) -> p two l", two=2)
    k8p = k8[:].rearrange("p (two l) -> p two l", two=2)
    v_sb, ho = [], []
    for t in range(NT):
        t_v = persist.tile([128, HPC * 65], BF16, tag=f"v{t}")
        v_sb.append(t_v)
    for j in range(2):
        t_ho = persist.tile([128, L], BF16, tag=f"ho{j}")
        ho.append(t_ho)

    def emit_rest_dmas(d):
        """One big DMA for the tail of xt stripe d (plus cs/sn for d 0/1);
        issued on the ACT sequencer so SP's latency-critical transposes can
        interleave on the shared HWDGE."""
        nc.scalar.dma_start(xt_sb[d][:, 2 * CH:], xt[128 * d:128 * d + 128, 2 * CH:])
        if d < 2:
            tbl_sb, tbl = (cs_sb, cs) if d == 0 else (sn_sb, sn)
            nc.scalar.dma_start(tbl_sb[:, 2 * CH:], tbl[:, 2 * CH:])

    def emit_proj_qk_mms(c, part, grp):
        """One 4-matmul group (2 d-steps) of the Q (part=0) / K (part=1)
        projection for chunk c into the shared pq psum tile."""
        pq = state["pq"]
        lsl = slice(CH * c, CH * (c + 1))
        for d in (2 * grp, 2 * grp + 1):
            for half in range(2):           # T, B
                csl = slice(256 * part + 128 * half, 256 * part + 128 * half + 128)
                nc.tensor.matmul(pq[:, 256 * half:256 * half + 256],
                                 wq_sb[d][:, csl], xt_sb[d][:, lsl],
                                 start=(d == 0), stop=(d == 7))

    def emit_rope(c, part):
        """RoPE from the pq psum tile into q8/k8 fp8 planes."""
        pq = state["pq"]
        lsl = slice(CH * c, CH * (c + 1))
        dst = q8p if part == 0 else k8p
        cs_c, sn_c = cs_sb[:, lsl], sn_sb[:, lsl]
        pt, pb = pq[:, 0:256], pq[:, 256:512]
        t1 = ropet.tile([128, CH], F32, tag="t1")
        t2 = ropet.tile([128, CH], F32, tag="t2")
        nc.vector.tensor_mul(t1, pt, cs_c)
        nc.vector.tensor_mul(t2, pb, sn_c)
        nc.vector.tensor_sub(dst[:, 0, lsl], t1, t2)
        t3 = ropet.tile([128, CH], F32, tag="t1")
        t4 = ropet.tile([128, CH], F32, tag="t2")
        nc.vector.tensor_mul(t3, pb, cs_c)
        nc.vector.tensor_mul(t4, pt, sn_c)
        nc.vector.tensor_add(dst[:, 1, lsl], t3, t4)

    def emit_v_tile(tv):
        """V projection for kv tile tv + copy into v_sb with ones column."""
        v_ps = ps.tile([128, 260], F32, tag=f"av{tv % 2}")
        for d in range(8):
            nc.tensor.matmul(v_ps[:, 0:256], xt_sb[d][:, 128 * tv:128 * tv + 128],
                             wv_sb[d][:], start=(d == 0), stop=(d == 7))
        vdst = v_sb[tv][:].rearrange("p (h x) -> p h x", x=65)[:, :, 0:64]
        vsrc = v_ps[:, 0:256].rearrange("p (h x) -> p h x", x=64)
        nc.gpsimd.tensor_copy(vdst, vsrc)
        nc.gpsimd.memset(v_sb[tv][:, 64:HPC * 65:65], 1.0)

    def emit_scores_exp(c, t):
        """DoubleRow scores for kv tile t of chunk c + exp + diag mask."""
        qsl = slice(CH * c, CH * (c + 1))
        ksl = slice(128 * t, 128 * t + 128)
        strip = ps.tile([128, 4 * CH], F32, tag=f"sc{t % 2}")
        for h in range(HPC):
            hsl = slice(32 * h, 32 * h + 32)
            nc.tensor.matmul(strip[:, 256 * h:256 * h + 256],
                             k8p[hsl, :, ksl], q8p[hsl, :, qsl],
                             start=True, stop=True, perf_mode=DR,
                             tile_position=(32 * h, 0))
        expt = epool.tile([128, 4 * CH], BF16, tag=f"e{t % 16}")
        off = 128 if t == 2 * c + 1 else 0
        esrc = strip[:].rearrange("kv (h q) -> kv h q", q=CH)[:, :, off:]
        edst = expt[:].rearrange("kv (h q) -> kv h q", q=CH)[:, :, off:]
        nc.scalar.activation(edst, esrc, AF.Exp, scale=0.125)
        if t >= 2 * c:  # diagonal tile: zero the upper triangle of its block
            for h in range(HPC):
                blk = slice(256 * h + off, 256 * h + off + 128)
                nc.gpsimd.affine_select(expt[:, blk], expt[:, blk],
                                        pattern=[[1, 128]],
                                        compare_op=mybir.AluOpType.is_ge,
                                        fill=0.0, base=0, channel_multiplier=-1)
        return expt

    def emit_av_norm(c, qt, expts):
        """AV accumulation for q-block qt of chunk c, then normalize and
        transpose (DMA) into ho layout."""
        lt = 2 * c + qt
        av_ps = ps.tile([128, 260], F32, tag=f"av{qt % 2}")
        ntile = 2 * c + qt + 1
        for t in range(ntile):  # t-major: only the last 4 mms wait on exp
            for h in range(HPC):
                nc.tensor.matmul(av_ps[:, 65 * h:65 * h + 65],
                                 expts[t][:, 256 * h + 128 * qt:256 * h + 128 * qt + 128],
                                 v_sb[t][:, 65 * h:65 * h + 65],
                                 start=(t == 0), stop=(t == ntile - 1))
        av_n = []
        for j in range(2):
            t_n = npool.tile([128, 128], BF16, tag=f"n{j}")
            av_n.append(t_n)
        for h in range(HPC):
            rec = rpool.tile([128, 1], F32, tag=f"r{h}")
            nc.vector.reciprocal(rec, av_ps[:, 65 * h + 64:65 * h + 65])
            nc.vector.tensor_scalar_mul(av_n[h // 2][:, 64 * (h % 2):64 * (h % 2) + 64],
                                        av_ps[:, 65 * h:65 * h + 64], rec)
        for j in range(2):
            nc.sync.dma_start_transpose(ho[j][:, 128 * lt:128 * lt + 128], av_n[j])

    def emit_outproj(lt):
        """Output projection + store for l-tile lt (ho rows already placed)."""
        last = lt == NT - 1
        o_sb = opool.tile([128, 1024], BF16, tag="o")
        for oc in range(2):
            op_ps = ps.tile([128, 512], F32, tag="op")
            for j in range(2):
                nc.tensor.matmul(op_ps, ho[j][:, 128 * lt:128 * lt + 128],
                                 wo_sb[j][:, 512 * oc:512 * oc + 512],
                                 start=(j == 0), stop=(j == 1))
            osl = slice(512 * oc, 512 * oc + 512)
            if last:
                # drain tail: fastest engines, store each half immediately
                if oc == 0:
                    nc.vector.tensor_copy(o_sb[:, osl], op_ps)
                else:
                    nc.scalar.copy(o_sb[:, osl], op_ps)
                nc.sync.dma_start(out[128 * lt:128 * lt + 128, osl], o_sb[:, osl])
            else:
                nc.gpsimd.tensor_copy(o_sb[:, osl], op_ps)
        if not last:
            nc.scalar.dma_start(out[128 * lt:128 * lt + 128, :], o_sb)

    def new_pq():
        t_pq = ps.tile([128, 512], F32, tag="pq")
        state["pq"] = t_pq

    # ---- preamble: project chunk 0 ----
    state = {}
    new_pq()
    for grp in range(4):
        emit_proj_qk_mms(0, 0, grp)
    emit_rope(0, 0)
    new_pq()
    for grp in range(4):
        emit_proj_qk_mms(0, 1, grp)
    emit_rope(0, 1)
    emit_v_tile(0)
    emit_v_tile(1)

    emit_scores_exp = _track(nc, "scores", emit_scores_exp)
    emit_av_norm = _track(nc, "av", emit_av_norm)
    emit_outproj = _track(nc, "outproj", emit_outproj)
    emit_v_tile = _track(nc, "vproj", emit_v_tile)
    emit_proj_qk_mms = _track(nc, "proj", emit_proj_qk_mms)
    emit_rope = _track(nc, "rope", emit_rope)

    # ---- main loop: per-tile scores/exp with a work queue of deferred PE
    # units (next-chunk projection, av batches, output projection) popped
    # between tiles so the PE never waits on a just-issued exp ----
    workq = []   # entries: (must_finish_this_chunk, fn)

    for c in range(NCH):
        if c + 1 < NCH:
            def mk_qk(cc, part, grp, last):
                def f():
                    if grp == 0:
                        new_pq()
                    emit_proj_qk_mms(cc, part, grp)
                    if last:
                        emit_rope(cc, part)
                return f
            for part in range(2):
                for grp in range(4):
                    workq.append((True, mk_qk(c + 1, part, grp, grp == 3)))
            workq.append((True, lambda cc=c: emit_v_tile(2 * (cc + 1))))
            workq.append((True, lambda cc=c: emit_v_tile(2 * (cc + 1) + 1)))
        if c < 2:
            for d in range(4 * c, 4 * c + 4):
                workq.append((True, lambda dd=d: emit_rest_dmas(dd)))

        expts = []
        for t in range(2 * c + 2):
            expts.append(emit_scores_exp(c, t))
            if workq:
                workq.pop(0)[1]()
            if t == 2 * c:
                workq.insert(0, (False, lambda cc=c, ee=expts: emit_av_norm(cc, 0, ee)))
                workq.insert(1, (False, lambda cc=c: emit_outproj(2 * cc)))
            elif t == 2 * c + 1:
                workq.insert(0, (False, lambda cc=c, ee=expts: emit_av_norm(cc, 1, ee)))
                workq.insert(2, (False, lambda cc=c: emit_outproj(2 * cc + 1)))
        # next-chunk projection (and rest-DMAs) must be in before its scores
        rest = []
        for must, fn in workq:
            if must:
                fn()
            else:
                rest.append((must, fn))
        workq = rest
    while workq:
        workq.pop(0)[1]()


def _build_nc():
    PHASE_RANGES.clear()
    nc = bacc.Bacc("TRN2", target_bir_lowering=False, debug=False,
                   enable_asserts=False, num_devices=8)
    ins = {
        "xt": nc.dram_tensor("xt", [DM, L], BF16, kind="ExternalInput").ap(),
        "wq": nc.dram_tensor("wq", [DM, 512], BF16, kind="ExternalInput").ap(),
        "wv": nc.dram_tensor("wv", [DM, 256], BF16, kind="ExternalInput").ap(),
        "wo": nc.dram_tensor("wo", [256, DM], BF16, kind="ExternalInput").ap(),
        "cs": nc.dram_tensor("cs", [128, L], BF16, kind="ExternalInput").ap(),
        "sn": nc.dram_tensor("sn", [128, L], BF16, kind="ExternalInput").ap(),
    }
    outs = {"out": nc.dram_tensor("out", [L, DM], BF16, kind="ExternalOutput").ap()}
    with tile.TileContext(nc) as tc:
        _attn_kernel(tc, outs, ins)
    nc.compile()
    return nc


def _host_shard(X, token_positions, Wqkv, Wout):
    """Build the 8 per-core input maps (bf16)."""
    import ml_dtypes
    bf = ml_dtypes.bfloat16
    X = np.asarray(X, dtype=np.float32)
    Wqkv = np.asarray(Wqkv, dtype=np.float32)
    Wout = np.asarray(Wout, dtype=np.float32)
    pos = np.asarray(token_positions)

    k = np.arange(DK // 2, dtype=np.float32)
    inv_freq = (np.float32(1.0) /
                np.power(np.float32(THETA), (np.float32(2.0) * k) / np.float32(DK)))
    ang = (pos.astype(np.float32)[:, None, :] *
           inv_freq.astype(np.float32)[None, :, None]).astype(np.float32)  # [B,32,L]
    cos = np.cos(ang).astype(np.float32)
    sin = np.sin(ang).astype(np.float32)
    cs_all = np.tile(cos, (1, HPC, 1)).astype(bf)  # [B, 128, L]
    sn_all = np.tile(sin, (1, HPC, 1)).astype(bf)

    in_maps = []
    for core in range(8):
        b, g = divmod(core, HPC)
        heads = [HPC * g + hh for hh in range(HPC)]
        q_top, q_bot, k_top, k_bot = [], [], [], []
        for h in heads:
            base = DK * h
            q_top += [base + 2 * kk for kk in range(DK // 2)]
            q_bot += [base + 2 * kk + 1 for kk in range(DK // 2)]
            k_top += [DM + base + 2 * kk for kk in range(DK // 2)]
            k_bot += [DM + base + 2 * kk + 1 for kk in range(DK // 2)]
        wq_c = np.ascontiguousarray(Wqkv[q_top + q_bot + k_top + k_bot, :].T.astype(bf))
        v_rows = [2 * DM + DK * h + j for h in heads for j in range(DK)]
        wv_c = np.ascontiguousarray(Wqkv[v_rows, :].T.astype(bf))
        wo_c = np.ascontiguousarray(Wout[:, 256 * g:256 * (g + 1)].T.astype(bf))
        in_maps.append({
            "xt": np.ascontiguousarray(X[b].T.astype(bf)),
            "wq": wq_c,
            "wv": wv_c,
            "wo": wo_c,
            "cs": np.ascontiguousarray(cs_all[b]),
            "sn": np.ascontiguousarray(sn_all[b]),
        })
    return in_maps


def kernel(X, token_positions, Wqkv, Wout, _trace=False):
    if "nc" not in _cache:
        _cache["nc"] = _build_nc()
    nc = _cache["nc"]
    in_maps = _host_shard(X, token_positions, Wqkv, Wout)
    res = run_bass_kernel_spmd(nc, in_maps, list(range(8)), trace=_trace)
    _cache["last_results"] = res
    out = np.zeros((B, L, DM), dtype=np.float32)
    for core in range(8):
        out[core // HPC] += np.asarray(res.results[core]["out"], dtype=np.float32)
    return out
